# revision 27
# baseline (speedup 1.0000x reference)
"""Trainium2 Bass kernel for nn_EnhancedFractionalPINO.

Pipeline (per core, batch-parallel over 8 NeuronCores, 32 batches/core):
  1. f = Re(fft2(x)) per 64x64 image via cosine/sine DFT matmuls:
     m1: per image, lhsT = image, rhs = [C | S] -> [x^T C | x^T S];
     m2: per 8-image group, two const-stationary matmuls with strided rhs
     -> A^T = C x^T C - S x^T S for all 8 images in one psum tile.
  2. GL fractional derivative = truncated causal conv (KTAPS taps) over the
     globally-flattened signal, as Toeplitz-block matmuls (halo image passed
     from the previous core's batch range; zeros for core 0). The h^-alpha
     scale is folded into Ws1 so everything stays in fp16 range.
  3. spectral_operator + neural_operator MLPs as fp16 PE matmuls in
     weights-stationary form: every layer's output is produced directly in
     transposed [feature-on-partition, batch-free] layout ([128, 32] psum
     slabs accumulated over the contraction), so no PE transposes are needed
     between layers and each matmul streams only 32 columns. Biases enter as
     rank-1 matmuls (lhsT = bias row, rhs = ones) or activation bias columns.
     A positive rescaling chain (LAM_*) keeps activations in fp16 range.
  4. W5 is column-permuted host-side (u<->v within each channel image) so L5
     psum slabs land directly in the [u-part, v, b] layout ifft2 wants.
     out = Re(ifft2(proc)) via the same DFT-matmul machinery; the final
     stage-2 psum tiles are DMA'd straight to DRAM in a partition-major
     layout (reassembled on host).

Weights are replicated across cores; activations stay SBUF-resident.
"""

import numpy as np

import concourse.bass as bass
import concourse.mybir as mybir
import concourse.tile as tile
from concourse import bacc
from concourse.bass_utils import run_bass_kernel_spmd

F32 = mybir.dt.float32
F16 = mybir.dt.float16
AF = mybir.ActivationFunctionType

B, C, H, W = 256, 3, 64, 64
MODES = C * H * W              # 12288
ALPHA = 0.5
NTOT = B * MODES               # 3145728 flattened samples
NCORE = 8
BS = B // NCORE                # 32 batches per core
NIMG = BS * C                  # 96 images per core
NSLOT = NIMG + 2               # halo + 96 images + zero pad
KTAPS = 512                    # truncated GL taps (4 chunks of 128)
NCH = BS * MODES // 128        # 3072 output chunks per core
NBLK = NCH // 512              # 6 conv blocks of 512 chunks
NK = MODES // 128              # 96 contraction blocks for the big layers

# fp16 activation rescaling chain (see mirror3 validation)
LAM_H, LAM_S, LAM_1, LAM_2, LAM_P = 16.0, 8.0, 4.0, 4.0, 4.0


# ---------------------------------------------------------------- host consts
def _host_constants():
    jk = np.outer(np.arange(64), np.arange(64)).astype(np.float64)
    Cm = np.cos(2 * np.pi * jk / 64)
    Sm = np.sin(2 * np.pi * jk / 64)

    j = np.arange(1, KTAPS, dtype=np.float64)
    w = np.concatenate([[1.0], np.cumprod((j - 1.0 - ALPHA) / j)])

    # Tst[d][t, tau] = w[128*d + tau - t]  (lhsT layout of the Toeplitz blocks)
    idx = 128 * np.arange(4)[:, None, None] \
        + np.arange(128)[None, None, :] - np.arange(128)[None, :, None]
    Tst = np.where((idx >= 0) & (idx < KTAPS), w[np.clip(idx, 0, KTAPS - 1)], 0.0)

    f16 = lambda a: np.ascontiguousarray(a, dtype=np.float16)
    return {
        "cswi": f16(np.concatenate([Cm, Sm], axis=1)),     # [64, 128]
        "cmf": f16(Cm),                                    # [64, 64]
        "msf": f16(-Sm),
        "cmi": f16(Cm * (LAM_P / 4096.0)),
        "smi": f16(-Sm * (LAM_P / 4096.0)),
        "tst": f16(Tst),
        "ones1": f16(np.ones((1, 32))),
    }


def _prep_weights(Ws1, bs1, Ws2, bs2, Wn1, bn1, Wn2, bn2, Wn3, bn3):
    s = float(np.float64(1.0 / (NTOT - 1)) ** (-ALPHA))
    f16 = lambda a: np.ascontiguousarray(a, dtype=np.float16)
    f32 = lambda a: np.ascontiguousarray(a, dtype=np.float32)
    W1 = (Ws1.astype(np.float64) * (s / LAM_H)).astype(np.float32)
    W2 = Ws2 * np.float32(LAM_H / LAM_S)
    W3 = Wn1 * np.float32(LAM_S / LAM_1)
    W4 = Wn2 * np.float32(LAM_1 / LAM_2)
    W5 = Wn3 * np.float32(LAM_2 / LAM_P)
    # permute W5/b5 output columns: within each channel image u<->v swap, so
    # psum chunk K = (c, v-pair) with partition q = 64*(v&1) + u
    perm = (np.arange(MODES).reshape(C, H, W).transpose(0, 2, 1)
            .reshape(-1))                                  # new col j <- old perm[j]
    W5p = W5[:, perm]
    b5p = (bn3 / LAM_P)[perm]
    return {
        # [12, 128, 8, 512]: lhsT for (K = 8g+i, j) = w1t[g][:, i, 128j:128j+128]
        "w1t": f16(W1.reshape(12, 8, 128, 512).transpose(0, 2, 1, 3)),
        # [12, 128, 4, 8, 128]: lhsT for (kk, K = 8g+Ki) = w2t[g][:, kk, Ki, :]
        "w2t": f16(W2.reshape(4, 128, 12, 8, 128).transpose(2, 1, 0, 3, 4)),
        "w3t": f16(W3.reshape(12, 8, 128, 512).transpose(0, 2, 1, 3)),
        # [128, 4, 4, 128]: lhsT for (kk, j) = w4t[:, kk, j, :]
        "w4t": f16(W4.reshape(4, 128, 4, 128).transpose(1, 0, 2, 3)),
        "w5t": f16(W5p.reshape(4, 128, 12, 8, 128).transpose(2, 1, 0, 3, 4)),
        "b1t": f32((bs1 / LAM_H).reshape(4, 128).T),       # [128, 4]
        "b2r": f16((bs2 / LAM_S).reshape(1, MODES)),
        "b3t": f32((bn1 / LAM_1).reshape(4, 128).T),
        "b4t": f32((bn2 / LAM_2).reshape(4, 128).T),
        "b5r": f16(b5p.reshape(1, MODES)),
    }


# ---------------------------------------------------------------- bass module
_NC_CACHE = None


def _build_nc():
    nc = bacc.Bacc("TRN2", target_bir_lowering=False, debug=False,
                   num_devices=NCORE)

    def din(name, shape, dt=F16):
        return nc.dram_tensor(name, shape, dt, kind="ExternalInput")

    d_x = din("ximgs", (64, NSLOT * 64))
    d_cswi = din("cswi", (64, 128))
    d_cmf = din("cmf", (64, 64))
    d_msf = din("msf", (64, 64))
    d_cmi = din("cmi", (64, 64))
    d_smi = din("smi", (64, 64))
    d_tst = din("tst", (4, 128, 128))
    d_ones = din("ones1", (1, 32))
    d_w1 = din("w1t", (12, 128, 8, 512))
    d_w2 = din("w2t", (12, 128, 4, 8, 128))
    d_w3 = din("w3t", (12, 128, 8, 512))
    d_w4 = din("w4t", (128, 4, 4, 128))
    d_w5 = din("w5t", (12, 128, 4, 8, 128))
    d_b1 = nc.dram_tensor("b1t", (128, 4), F32, kind="ExternalInput")
    d_b2 = din("b2r", (1, MODES))
    d_b3 = nc.dram_tensor("b3t", (128, 4), F32, kind="ExternalInput")
    d_b4 = nc.dram_tensor("b4t", (128, 4), F32, kind="ExternalInput")
    d_b5 = din("b5r", (1, MODES))
    # [u2, (c, bg, t, v)] partition-major f32; host reassembles
    d_out = nc.dram_tensor("out", (64, NIMG * 64), F16, kind="ExternalOutput")
    import os
    dbg = os.environ.get("KERNEL_DEBUG_DUMPS") == "1"
    if dbg:
        d_dbg = {n: nc.dram_tensor(f"dbg_{n}", shp, F16, kind="ExternalOutput")
                 for n, shp in (("hT", (128, 4, BS)), ("h1T", (128, 4, BS)),
                                ("h2T", (128, 4, BS)), ("specT", (128, NK, BS)),
                                ("procT0", (64, 64, BS)),
                                ("frlin", (128, NCH)))}

    with tile.TileContext(nc) as tc:
        # weight pools opened up-front so their SBUF ranges are disjoint from
        # the fft2-phase pools: the weight DMA stream then has no false deps
        # and starts at t=0, keeping the DMA engines saturated end-to-end.
        with tc.tile_pool(name="cpool", bufs=1) as cpool, \
             tc.tile_pool(name="bigpool", bufs=1) as bigpool, \
             tc.tile_pool(name="wpool", bufs=14) as wpool, \
             tc.tile_pool(name="wp4", bufs=1) as wp4:
            # ---- constants into SBUF
            cswi = cpool.tile([64, 128], F16, tag="cswi")
            cmf = cpool.tile([64, 64], F16, tag="cmf")
            msf = cpool.tile([64, 64], F16, tag="msf")
            cmi = cpool.tile([64, 64], F16, tag="cmi")
            smi = cpool.tile([64, 64], F16, tag="smi")
            tsb = cpool.tile([128, 4, 128], F16, tag="tsb")
            ones1 = cpool.tile([1, 32], F16, tag="ones1")
            b1s = cpool.tile([128, 4], F32, tag="b1s")
            b3s = cpool.tile([128, 4], F32, tag="b3s")
            b4s = cpool.tile([128, 4], F32, tag="b4s")
            bbig = cpool.tile([1, MODES], F16, tag="bbig")  # b2 then b5
            for t, d in ((cswi, d_cswi), (cmf, d_cmf), (msf, d_msf),
                         (cmi, d_cmi), (smi, d_smi), (ones1, d_ones),
                         (b1s, d_b1), (b3s, d_b3), (b4s, d_b4)):
                nc.gpsimd.dma_start(t[:], d[:])
            nc.gpsimd.dma_start(tsb[:], d_tst.rearrange("d p k -> p d k"))
            nc.gpsimd.dma_start(bbig[:], d_b2[:])

            # ---- persistent activation tiles
            fbuf = bigpool.tile([128, 4 + NCH + 64], F16, tag="fbuf")
            frlin = bigpool.tile([128, NCH], F16, tag="frlin")
            specT = bigpool.tile([128, NK, BS], F16, tag="specT")
            procTs = [bigpool.tile([64, 64, BS], F16, tag=f"procT{i}",
                                   name=f"procT{i}") for i in range(C)]
            hT = bigpool.tile([128, 4, BS], F16, tag="hT")
            h1T = bigpool.tile([128, 4, BS], F16, tag="h1T")
            h2T = bigpool.tile([128, 4, BS], F16, tag="h2T")

            # ========== phase 1: fft2 (per-image m1, 8-wide m2) =============
            with tc.tile_pool(name="xpool", bufs=1) as xpool, \
                 tc.tile_pool(name="gpool", bufs=6) as gpool, \
                 tc.tile_pool(name="ps1p", bufs=4, space="PSUM") as ps1p, \
                 tc.tile_pool(name="ps2p", bufs=3, space="PSUM") as ps2p:
                xall = xpool.tile([64, NSLOT, 64], F16, tag="xall")
                nc.scalar.dma_start(
                    xall.rearrange("p q k -> p (q k)"), d_x[:])
                for grp in range(25):
                    n = 4 if grp < 24 else 2
                    psA = ps1p.tile([64, 512], F32, tag="psA")
                    for t in range(n):
                        i = grp * 4 + t
                        nc.tensor.matmul(psA[:, t * 128:(t + 1) * 128],
                                         xall[:, i, :], cswi[:],
                                         start=True, stop=True)
                    g1w = gpool.tile([64, 4, 128], F16, tag="g1w")
                    g1f = g1w[:, 0:n, :].rearrange("p a k -> p (a k)")
                    if grp % 2 == 0:
                        nc.scalar.copy(g1f, psA[:, 0:n * 128])
                    else:
                        nc.vector.tensor_copy(g1f, psA[:, 0:n * 128])
                    ps2 = ps2p.tile([64, 256], F32, tag="ps2")
                    nc.tensor.matmul(ps2[:, 0:n * 64], cmf[:],
                                     g1w[:, 0:n, 0:64], start=True, stop=False)
                    nc.tensor.matmul(ps2[:, 0:n * 64], msf[:],
                                     g1w[:, 0:n, 64:128], start=False, stop=True)
                    p2v = ps2.rearrange("p (k two) -> p k two", two=2)
                    if grp == 0:
                        # halo image: last 4 chunk-cols; imgs 1..3 -> cols 4:100
                        nc.vector.tensor_copy(fbuf[0:64, 0:4], p2v[:, 28:32, 0])
                        nc.vector.tensor_copy(fbuf[64:128, 0:4], p2v[:, 28:32, 1])
                        nc.vector.tensor_copy(fbuf[0:64, 4:100], p2v[:, 32:128, 0])
                        nc.vector.tensor_copy(fbuf[64:128, 4:100],
                                              p2v[:, 32:128, 1])
                    else:
                        base = 4 + (grp * 4 - 1) * 32
                        nc.vector.tensor_copy(fbuf[0:64, base:base + n * 32],
                                              p2v[:, 0:n * 32, 0])
                        nc.vector.tensor_copy(fbuf[64:128, base:base + n * 32],
                                              p2v[:, 0:n * 32, 1])

            # ================= phase 2: conv ================================
            with tc.tile_pool(name="pscv2", bufs=1, space="PSUM") as pscv2:
                psc = [pscv2.tile([128, 512], F32, tag=f"psc{i}",
                                  name=f"psc{i}") for i in range(NBLK)]
                for d in range(4):
                    for blk in range(NBLK):
                        o = 4 + blk * 512 - d
                        nc.tensor.matmul(psc[blk][:], tsb[:, d, :],
                                         fbuf[:, o:o + 512],
                                         start=(d == 0), stop=(d == 3))
                for blk in range(NBLK):
                    nc.vector.tensor_copy(frlin[:, blk * 512:(blk + 1) * 512],
                                          psc[blk][:])

            # frT k-block slices: frl3[:, K, :] = [128, 32] (stride-96 free)
            frl3 = frlin.rearrange("p (b k) -> p k b", b=BS)

            # ======= L1: 12288 -> 512, weights-stationary ===================
            # Concurrently-open accumulation groups must live in separate
            # PSUM banks (interleaved groups within one bank corrupt), so
            # each output chunk j gets its own full-bank tile.
            with tc.tile_pool(name="psm1", bufs=1, space="PSUM") as psm1:
                psH = [psm1.tile([128, 512], F32, tag=f"psH{j}",
                                 name=f"psH{j}") for j in range(4)]
                for g in range(12):
                    wt = wpool.tile([128, 4096], F16, tag="w", name="w").rearrange(
                        "p (i k) -> p i k", i=8)
                    nc.sync.dma_start(wt[:], d_w1[g])
                    for i in range(8):
                        K = 8 * g + i
                        for j in range(4):
                            nc.tensor.matmul(
                                psH[j][:, 0:BS],
                                wt[:, i, 128 * j:128 * (j + 1)],
                                frl3[:, K, :],
                                start=(K == 0), stop=(K == NK - 1))
                for j in range(4):
                    nc.scalar.activation(hT[:, j, :], psH[j][:, 0:BS],
                                         AF.Relu, bias=b1s[:, j:j + 1])

            # ======= L2 + L3 interleaved ====================================
            # L2 produces specT chunks 8g..8g+7 from weight tile g; L3's
            # contraction blocks 8g..8g+7 consume exactly those chunks, so
            # both ride the same loop with independent psum accumulators.
            with tc.tile_pool(name="ps2m", bufs=3, space="PSUM") as ps2m, \
                 tc.tile_pool(name="psm3", bufs=1, space="PSUM") as psm3:
                psH1 = [psm3.tile([128, 512], F32, tag=f"psH1{j}",
                                  name=f"psH1{j}") for j in range(4)]
                for g in range(12):
                    wt2 = wpool.tile([128, 4096], F16, tag="w", name="w").rearrange(
                        "p (a b c) -> p a b c", a=4, b=8)
                    nc.sync.dma_start(wt2[:], d_w2[g])
                    wt3 = wpool.tile([128, 4096], F16, tag="w", name="w").rearrange(
                        "p (i k) -> p i k", i=8)
                    nc.sync.dma_start(wt3[:], d_w3[g])
                    psS = ps2m.tile([128, 8, BS], F32, tag="psS")
                    for Ki in range(8):
                        K = 8 * g + Ki
                        for kk in range(4):
                            nc.tensor.matmul(psS[:, Ki, :],
                                             wt2[:, kk, Ki, :],
                                             hT[:, kk, :],
                                             start=(kk == 0), stop=False)
                        nc.tensor.matmul(psS[:, Ki, :],
                                         bbig[0:1, 128 * K:128 * (K + 1)],
                                         ones1[:], start=False, stop=True)
                    if g % 2 == 0:
                        nc.scalar.copy(specT[:, 8 * g:8 * (g + 1), :],
                                       psS[:, :, :])
                    else:
                        nc.vector.tensor_copy(specT[:, 8 * g:8 * (g + 1), :],
                                              psS[:, :, :])
                    for i in range(8):
                        K = 8 * g + i
                        for j in range(4):
                            nc.tensor.matmul(
                                psH1[j][:, 0:BS],
                                wt3[:, i, 128 * j:128 * (j + 1)],
                                specT[:, K, :],
                                start=(K == 0), stop=(K == NK - 1))
                for j in range(4):
                    nc.scalar.activation(h1T[:, j, :], psH1[j][:, 0:BS],
                                         AF.Relu, bias=b3s[:, j:j + 1])

            nc.gpsimd.dma_start(bbig[:], d_b5[:])

            # ======= L4: 512 -> 512, single weight tile =====================
            with tc.tile_pool(name="psm4", bufs=1, space="PSUM") as psm4:
                w4 = wp4.tile([128, 4, 4, 128], F16, tag="w4")
                nc.sync.dma_start(w4[:], d_w4[:])
                psH2 = psm4.tile([128, 4, BS], F32, tag="psH2")
                for j in range(4):
                    for kk in range(4):
                        nc.tensor.matmul(psH2[:, j, :],
                                         w4[:, kk, j, :],
                                         h1T[:, kk, :],
                                         start=(kk == 0), stop=(kk == 3))
                    nc.scalar.activation(h2T[:, j, :], psH2[:, j, :], AF.Relu,
                                         bias=b4s[:, j:j + 1])

            # ======= L5 + ifft2, emission-interleaved by channel ============
            # psum chunk K = (c = K//32, v-pair): partition q = 64*(v&1) + u
            # lands psP[0:64] -> even v columns, psP[64:128] -> odd v.
            with tc.tile_pool(name="gpi", bufs=6) as gpi, \
                 tc.tile_pool(name="opool", bufs=6) as opool, \
                 tc.tile_pool(name="ps5m", bufs=1, space="PSUM") as ps5m, \
                 tc.tile_pool(name="ps1i", bufs=4, space="PSUM") as ps1i, \
                 tc.tile_pool(name="ps2i", bufs=3, space="PSUM") as ps2i:

                ot_box = [None]

                def ifft2_bg(c, bg, flush=False):
                    psA = ps1i.tile([64, 512], F32, tag="psAi", name="psAi")
                    for t in range(4):
                        b = bg * 4 + t
                        nc.tensor.matmul(psA[:, t * 128:(t + 1) * 128],
                                         procTs[c][:, :, b],
                                         cswi[:], start=True, stop=True)
                    g1w = gpi.tile([64, 4, 128], F16, tag="g1i", name="g1i")
                    if bg % 2 == 0:
                        nc.scalar.copy(g1w.rearrange("p a k -> p (a k)"),
                                       psA[:])
                    else:
                        nc.vector.tensor_copy(
                            g1w.rearrange("p a k -> p (a k)"), psA[:])
                    ps2 = ps2i.tile([64, 256], F32, tag="p2i", name="p2i")
                    nc.tensor.matmul(ps2[:], cmi[:], g1w[:, :, 0:64],
                                     start=True, stop=False)
                    nc.tensor.matmul(ps2[:], smi[:], g1w[:, :, 64:128],
                                     start=False, stop=True)
                    # two bg's share one fp16 staging tile -> one DMA;
                    # staging copies ride the otherwise-idle gpsimd engine,
                    # and the post-stream flush DMAs use the freed SP queue.
                    if bg % 2 == 0:
                        ot_box[0] = opool.tile([64, 2, 256], F16, tag="ot",
                                               name="ot")
                        nc.vector.tensor_copy(ot_box[0][:, 0, :], ps2[:])
                    else:
                        nc.scalar.copy(ot_box[0][:, 1, :], ps2[:])
                        eng = nc.sync if flush else nc.gpsimd
                        eng.dma_start(
                            d_out[:, (c * 8 + bg - 1) * 256:
                                  (c * 8 + bg + 1) * 256],
                            ot_box[0].rearrange("p a k -> p (a k)"))

                # ifft2 image-groups of channel c are spread 2-per-slab
                # across the next channel's L5 slabs so their copy waits
                # overlap slab matmuls; the last channel flushes at the end.
                pending = []
                for g in range(12):
                    c, gc = g // 4, g % 4
                    wt5 = wpool.tile([128, 4096], F16, tag="w", name="w").rearrange(
                        "p (a b c) -> p a b c", a=4, b=8)
                    nc.sync.dma_start(wt5[:], d_w5[g])
                    psP = ps5m.tile([128, 8, BS], F32, tag="psP")
                    for Ki in range(8):
                        K = 8 * g + Ki
                        for kk in range(4):
                            nc.tensor.matmul(psP[:, Ki, :],
                                             wt5[:, kk, Ki, :],
                                             h2T[:, kk, :],
                                             start=(kk == 0), stop=False)
                        nc.tensor.matmul(psP[:, Ki, :],
                                         bbig[0:1, 128 * K:128 * (K + 1)],
                                         ones1[:], start=False, stop=True)
                    v0 = gc * 16
                    if g % 2 == 0:
                        nc.scalar.copy(procTs[c][:, v0:v0 + 16:2, :],
                                       psP[0:64, :, :])
                        nc.vector.tensor_copy(
                            procTs[c][:, v0 + 1:v0 + 16:2, :],
                            psP[64:128, :, :])
                    else:
                        nc.vector.tensor_copy(procTs[c][:, v0:v0 + 16:2, :],
                                              psP[0:64, :, :])
                        nc.scalar.copy(procTs[c][:, v0 + 1:v0 + 16:2, :],
                                       psP[64:128, :, :])
                    for cb in pending[:3]:
                        ifft2_bg(*cb)
                    pending = pending[3:]
                    if gc == 3:
                        pending += [(c, bg) for bg in range(BS // 4)]
                for cb in pending:
                    ifft2_bg(*cb, flush=True)
                if dbg:
                    for t, n in ((hT, "hT"), (h1T, "h1T"), (h2T, "h2T"),
                                 (specT, "specT"), (procTs[0], "procT0"),
                                 (frlin, "frlin")):
                        nc.scalar.dma_start(d_dbg[n][:], t[:])

    nc.compile()
    return nc


def _get_nc():
    global _NC_CACHE
    if _NC_CACHE is None:
        _NC_CACHE = _build_nc()
    return _NC_CACHE


def _make_in_maps(x, Ws1, bs1, Ws2, bs2, Wn1, bn1, Wn2, bn2, Wn3, bn3):
    shared = dict(_host_constants())
    shared.update(_prep_weights(Ws1, bs1, Ws2, bs2, Wn1, bn1, Wn2, bn2,
                                Wn3, bn3))
    in_maps = []
    for g in range(NCORE):
        if g == 0:
            halo = np.zeros((1, 64, 64), np.float32)
        else:
            halo = x[g * BS - 1, 2][None]
        ximgs = np.concatenate(
            [halo, x[g * BS:(g + 1) * BS].reshape(NIMG, 64, 64),
             np.zeros((1, 64, 64), np.float32)]).astype(np.float16)
        # pre-transpose to [u, (slot, v)] so the device DMA is contiguous
        xT = np.ascontiguousarray(ximgs.transpose(1, 0, 2).reshape(64, -1))
        in_maps.append({"ximgs": xT, **shared})
    return in_maps


def _assemble(per_core_outs):
    """per_core_outs[g]: [64, NIMG*64] f32; row = v', col = (c*32 + b)*64 + u'
    (procT is [u-part, v-free], so the ifft2 DFT pair contracts u first and
    stage-2 psum comes out image-transposed)."""
    out = np.empty((B, C, H, W), np.float32)
    for g in range(NCORE):
        a = np.asarray(per_core_outs[g]).astype(np.float32).reshape(
            64, C, BS, 64)
        out[g * BS:(g + 1) * BS] = a.transpose(2, 1, 3, 0)
    return out


def kernel(**inputs):
    x = np.ascontiguousarray(inputs["x"], dtype=np.float32)
    nc = _get_nc()
    in_maps = _make_in_maps(
        x, inputs["Ws1"], inputs["bs1"], inputs["Ws2"], inputs["bs2"],
        inputs["Wn1"], inputs["bn1"], inputs["Wn2"], inputs["bn2"],
        inputs["Wn3"], inputs["bn3"])
    res = run_bass_kernel_spmd(nc, in_maps, list(range(NCORE)))
    return _assemble([res.results[g]["out"] for g in range(NCORE)])


# revision 28
# speedup vs baseline: 1.0336x; 1.0336x over previous
"""Trainium2 Bass kernel for nn_EnhancedFractionalPINO.

Pipeline (per core, batch-parallel over 8 NeuronCores, 32 batches/core):
  1. f = Re(fft2(x)) per 64x64 image via cosine/sine DFT matmuls:
     m1: per image, lhsT = image, rhs = [C | S] -> [x^T C | x^T S];
     m2: per 8-image group, two const-stationary matmuls with strided rhs
     -> A^T = C x^T C - S x^T S for all 8 images in one psum tile.
  2. GL fractional derivative = truncated causal conv (KTAPS taps) over the
     globally-flattened signal, as Toeplitz-block matmuls (halo image passed
     from the previous core's batch range; zeros for core 0). The h^-alpha
     scale is folded into Ws1 so everything stays in fp16 range.
  3. spectral_operator + neural_operator MLPs as fp16 PE matmuls in
     weights-stationary form: every layer's output is produced directly in
     transposed [feature-on-partition, batch-free] layout ([128, 32] psum
     slabs accumulated over the contraction), so no PE transposes are needed
     between layers and each matmul streams only 32 columns. Biases enter as
     rank-1 matmuls (lhsT = bias row, rhs = ones) or activation bias columns.
     A positive rescaling chain (LAM_*) keeps activations in fp16 range.
  4. W5 is column-permuted host-side (u<->v within each channel image) so L5
     psum slabs land directly in the [u-part, v, b] layout ifft2 wants.
     out = Re(ifft2(proc)) via the same DFT-matmul machinery; the final
     stage-2 psum tiles are DMA'd straight to DRAM in a partition-major
     layout (reassembled on host).

Weights are replicated across cores; activations stay SBUF-resident.
"""

import numpy as np

import concourse.bass as bass
import concourse.mybir as mybir
import concourse.tile as tile
from concourse import bacc
from concourse.bass_utils import run_bass_kernel_spmd

F32 = mybir.dt.float32
F16 = mybir.dt.float16
AF = mybir.ActivationFunctionType

B, C, H, W = 256, 3, 64, 64
MODES = C * H * W              # 12288
ALPHA = 0.5
NTOT = B * MODES               # 3145728 flattened samples
NCORE = 8
BS = B // NCORE                # 32 batches per core
NIMG = BS * C                  # 96 images per core
NSLOT = NIMG + 2               # halo + 96 images + zero pad
KTAPS = 512                    # truncated GL taps (4 chunks of 128)
NCH = BS * MODES // 128        # 3072 output chunks per core
NBLK = NCH // 512              # 6 conv blocks of 512 chunks
NK = MODES // 128              # 96 contraction blocks for the big layers

# fp16 activation rescaling chain (see mirror3 validation)
LAM_H, LAM_S, LAM_1, LAM_2, LAM_P = 16.0, 8.0, 4.0, 4.0, 4.0


# ---------------------------------------------------------------- host consts
def _host_constants():
    jk = np.outer(np.arange(64), np.arange(64)).astype(np.float64)
    Cm = np.cos(2 * np.pi * jk / 64)
    Sm = np.sin(2 * np.pi * jk / 64)

    j = np.arange(1, KTAPS, dtype=np.float64)
    w = np.concatenate([[1.0], np.cumprod((j - 1.0 - ALPHA) / j)])

    # Tst[d][t, tau] = w[128*d + tau - t]  (lhsT layout of the Toeplitz blocks)
    idx = 128 * np.arange(4)[:, None, None] \
        + np.arange(128)[None, None, :] - np.arange(128)[None, :, None]
    Tst = np.where((idx >= 0) & (idx < KTAPS), w[np.clip(idx, 0, KTAPS - 1)], 0.0)

    f16 = lambda a: np.ascontiguousarray(a, dtype=np.float16)
    return {
        "cswi": f16(np.concatenate([Cm, Sm], axis=1)),     # [64, 128]
        "cmf": f16(Cm),                                    # [64, 64]
        "msf": f16(-Sm),
        "cmi": f16(Cm * (LAM_P / 4096.0)),
        "smi": f16(-Sm * (LAM_P / 4096.0)),
        "tst": f16(Tst),
        "ones1": f16(np.ones((1, 32))),
    }


def _prep_weights(Ws1, bs1, Ws2, bs2, Wn1, bn1, Wn2, bn2, Wn3, bn3):
    s = float(np.float64(1.0 / (NTOT - 1)) ** (-ALPHA))
    f16 = lambda a: np.ascontiguousarray(a, dtype=np.float16)
    f32 = lambda a: np.ascontiguousarray(a, dtype=np.float32)
    W1 = (Ws1.astype(np.float64) * (s / LAM_H)).astype(np.float32)
    W2 = Ws2 * np.float32(LAM_H / LAM_S)
    W3 = Wn1 * np.float32(LAM_S / LAM_1)
    W4 = Wn2 * np.float32(LAM_1 / LAM_2)
    W5 = Wn3 * np.float32(LAM_2 / LAM_P)
    # permute W5/b5 output columns: within each channel image u<->v swap, so
    # psum chunk K = (c, v-pair) with partition q = 64*(v&1) + u
    perm = (np.arange(MODES).reshape(C, H, W).transpose(0, 2, 1)
            .reshape(-1))                                  # new col j <- old perm[j]
    W5p = W5[:, perm]
    b5p = (bn3 / LAM_P)[perm]
    return {
        # [12, 128, 8, 512]: lhsT for (K = 8g+i, j) = w1t[g][:, i, 128j:128j+128]
        "w1t": f16(W1.reshape(12, 8, 128, 512).transpose(0, 2, 1, 3)),
        # [12, 128, 4, 8, 128]: lhsT for (kk, K = 8g+Ki) = w2t[g][:, kk, Ki, :]
        "w2t": f16(W2.reshape(4, 128, 12, 8, 128).transpose(2, 1, 0, 3, 4)),
        "w3t": f16(W3.reshape(12, 8, 128, 512).transpose(0, 2, 1, 3)),
        # [128, 4, 4, 128]: lhsT for (kk, j) = w4t[:, kk, j, :]
        "w4t": f16(W4.reshape(4, 128, 4, 128).transpose(1, 0, 2, 3)),
        "w5t": f16(W5p.reshape(4, 128, 12, 8, 128).transpose(2, 1, 0, 3, 4)),
        "b1t": f32((bs1 / LAM_H).reshape(4, 128).T),       # [128, 4]
        "b2r": f16((bs2 / LAM_S).reshape(1, MODES)),
        "b3t": f32((bn1 / LAM_1).reshape(4, 128).T),
        "b4t": f32((bn2 / LAM_2).reshape(4, 128).T),
        "b5r": f16(b5p.reshape(1, MODES)),
    }


# ---------------------------------------------------------------- bass module
_NC_CACHE = None


def _build_nc():
    nc = bacc.Bacc("TRN2", target_bir_lowering=False, debug=False,
                   num_devices=NCORE)

    def din(name, shape, dt=F16):
        return nc.dram_tensor(name, shape, dt, kind="ExternalInput")

    d_x = din("ximgs", (64, NSLOT * 64))
    d_cswi = din("cswi", (64, 128))
    d_cmf = din("cmf", (64, 64))
    d_msf = din("msf", (64, 64))
    d_cmi = din("cmi", (64, 64))
    d_smi = din("smi", (64, 64))
    d_tst = din("tst", (4, 128, 128))
    d_ones = din("ones1", (1, 32))
    d_w1 = din("w1t", (12, 128, 8, 512))
    d_w2 = din("w2t", (12, 128, 4, 8, 128))
    d_w3 = din("w3t", (12, 128, 8, 512))
    d_w4 = din("w4t", (128, 4, 4, 128))
    d_w5 = din("w5t", (12, 128, 4, 8, 128))
    d_b1 = nc.dram_tensor("b1t", (128, 4), F32, kind="ExternalInput")
    d_b2 = din("b2r", (1, MODES))
    d_b3 = nc.dram_tensor("b3t", (128, 4), F32, kind="ExternalInput")
    d_b4 = nc.dram_tensor("b4t", (128, 4), F32, kind="ExternalInput")
    d_b5 = din("b5r", (1, MODES))
    # [u2, (c, bg, t, v)] partition-major f32; host reassembles
    d_out = nc.dram_tensor("out", (64, NIMG * 64), F16, kind="ExternalOutput")
    import os
    dbg = os.environ.get("KERNEL_DEBUG_DUMPS") == "1"
    if dbg:
        d_dbg = {n: nc.dram_tensor(f"dbg_{n}", shp, F16, kind="ExternalOutput")
                 for n, shp in (("hT", (128, 4, BS)), ("h1T", (128, 4, BS)),
                                ("h2T", (128, 4, BS)), ("specT", (128, NK, BS)),
                                ("procT0", (64, 64, BS)),
                                ("frlin", (128, NCH)))}

    with tile.TileContext(nc) as tc:
        # weight pools opened up-front so their SBUF ranges are disjoint from
        # the fft2-phase pools: the weight DMA stream then has no false deps
        # and starts at t=0, keeping the DMA engines saturated end-to-end.
        with tc.tile_pool(name="cpool", bufs=1) as cpool, \
             tc.tile_pool(name="bigpool", bufs=1) as bigpool, \
             tc.tile_pool(name="wpool", bufs=14) as wpool, \
             tc.tile_pool(name="wp4", bufs=1) as wp4:
            # ---- constants into SBUF
            cswi = cpool.tile([64, 128], F16, tag="cswi")
            cmf = cpool.tile([64, 64], F16, tag="cmf")
            msf = cpool.tile([64, 64], F16, tag="msf")
            cmi = cpool.tile([64, 64], F16, tag="cmi")
            smi = cpool.tile([64, 64], F16, tag="smi")
            tsb = cpool.tile([128, 4, 128], F16, tag="tsb")
            ones1 = cpool.tile([1, 32], F16, tag="ones1")
            b1s = cpool.tile([128, 4], F32, tag="b1s")
            b3s = cpool.tile([128, 4], F32, tag="b3s")
            b4s = cpool.tile([128, 4], F32, tag="b4s")
            bbig = cpool.tile([1, MODES], F16, tag="bbig")  # b2 then b5
            for t, d in ((cswi, d_cswi), (cmf, d_cmf), (msf, d_msf),
                         (cmi, d_cmi), (smi, d_smi), (ones1, d_ones),
                         (b1s, d_b1), (b3s, d_b3), (b4s, d_b4)):
                nc.gpsimd.dma_start(t[:], d[:])
            nc.gpsimd.dma_start(tsb[:], d_tst.rearrange("d p k -> p d k"))
            nc.gpsimd.dma_start(bbig[:], d_b2[:])

            # ---- persistent activation tiles
            fbuf = bigpool.tile([128, 4 + NCH + 64], F16, tag="fbuf")
            frlin = bigpool.tile([128, NCH], F16, tag="frlin")
            specT = bigpool.tile([128, NK, BS], F16, tag="specT")
            procTs = [bigpool.tile([64, 64, BS], F16, tag=f"procT{i}",
                                   name=f"procT{i}") for i in range(C)]
            hT = bigpool.tile([128, 4, BS], F16, tag="hT")
            h1T = bigpool.tile([128, 4, BS], F16, tag="h1T")
            h2T = bigpool.tile([128, 4, BS], F16, tag="h2T")

            # ========== phase 1: fft2 (per-image m1, 8-wide m2) =============
            with tc.tile_pool(name="xpool", bufs=1) as xpool, \
                 tc.tile_pool(name="gpool", bufs=6) as gpool, \
                 tc.tile_pool(name="ps1p", bufs=4, space="PSUM") as ps1p, \
                 tc.tile_pool(name="ps2p", bufs=3, space="PSUM") as ps2p:
                xall = xpool.tile([64, NSLOT, 64], F16, tag="xall")
                nc.scalar.dma_start(
                    xall.rearrange("p q k -> p (q k)"), d_x[:])
                for grp in range(25):
                    n = 4 if grp < 24 else 2
                    psA = ps1p.tile([64, 512], F32, tag="psA")
                    for t in range(n):
                        i = grp * 4 + t
                        nc.tensor.matmul(psA[:, t * 128:(t + 1) * 128],
                                         xall[:, i, :], cswi[:],
                                         start=True, stop=True)
                    g1w = gpool.tile([64, 4, 128], F16, tag="g1w")
                    g1f = g1w[:, 0:n, :].rearrange("p a k -> p (a k)")
                    if grp % 2 == 0:
                        nc.scalar.copy(g1f, psA[:, 0:n * 128])
                    else:
                        nc.vector.tensor_copy(g1f, psA[:, 0:n * 128])
                    ps2 = ps2p.tile([64, 256], F32, tag="ps2")
                    nc.tensor.matmul(ps2[:, 0:n * 64], cmf[:],
                                     g1w[:, 0:n, 0:64], start=True, stop=False)
                    nc.tensor.matmul(ps2[:, 0:n * 64], msf[:],
                                     g1w[:, 0:n, 64:128], start=False, stop=True)
                    p2v = ps2.rearrange("p (k two) -> p k two", two=2)
                    if grp == 0:
                        # halo image: last 4 chunk-cols; imgs 1..3 -> cols 4:100
                        nc.vector.tensor_copy(fbuf[0:64, 0:4], p2v[:, 28:32, 0])
                        nc.vector.tensor_copy(fbuf[64:128, 0:4], p2v[:, 28:32, 1])
                        nc.vector.tensor_copy(fbuf[0:64, 4:100], p2v[:, 32:128, 0])
                        nc.vector.tensor_copy(fbuf[64:128, 4:100],
                                              p2v[:, 32:128, 1])
                    else:
                        base = 4 + (grp * 4 - 1) * 32
                        nc.vector.tensor_copy(fbuf[0:64, base:base + n * 32],
                                              p2v[:, 0:n * 32, 0])
                        nc.vector.tensor_copy(fbuf[64:128, base:base + n * 32],
                                              p2v[:, 0:n * 32, 1])

            # ================= phase 2: conv ================================
            with tc.tile_pool(name="pscv2", bufs=1, space="PSUM") as pscv2:
                psc = [pscv2.tile([128, 512], F32, tag=f"psc{i}",
                                  name=f"psc{i}") for i in range(NBLK)]
                for d in range(4):
                    for blk in range(NBLK):
                        o = 4 + blk * 512 - d
                        nc.tensor.matmul(psc[blk][:], tsb[:, d, :],
                                         fbuf[:, o:o + 512],
                                         start=(d == 0), stop=(d == 3))
                for blk in range(NBLK):
                    nc.vector.tensor_copy(frlin[:, blk * 512:(blk + 1) * 512],
                                          psc[blk][:])

            # frT k-block slices: frl3[:, K, :] = [128, 32] (stride-96 free)
            frl3 = frlin.rearrange("p (b k) -> p k b", b=BS)

            # ======= L1: 12288 -> 512, weights-stationary ===================
            # Concurrently-open accumulation groups must live in separate
            # PSUM banks (interleaved groups within one bank corrupt), so
            # each output chunk j gets its own full-bank tile.
            with tc.tile_pool(name="psm1", bufs=1, space="PSUM") as psm1:
                psH = [psm1.tile([128, 512], F32, tag=f"psH{j}",
                                 name=f"psH{j}") for j in range(4)]
                for g in range(12):
                    wt = wpool.tile([128, 4096], F16, tag="w", name="w").rearrange(
                        "p (i k) -> p i k", i=8)
                    nc.sync.dma_start(wt[:], d_w1[g])
                    for i in range(8):
                        K = 8 * g + i
                        for j in range(4):
                            nc.tensor.matmul(
                                psH[j][:, 0:BS],
                                wt[:, i, 128 * j:128 * (j + 1)],
                                frl3[:, K, :],
                                start=(K == 0), stop=(K == NK - 1))
                for j in range(4):
                    nc.scalar.activation(hT[:, j, :], psH[j][:, 0:BS],
                                         AF.Relu, bias=b1s[:, j:j + 1])

            # ======= L2 + L3 interleaved ====================================
            # L2 produces specT chunks 8g..8g+7 from weight tile g; L3's
            # contraction blocks 8g..8g+7 consume exactly those chunks, so
            # both ride the same loop with independent psum accumulators.
            with tc.tile_pool(name="ps2m", bufs=3, space="PSUM") as ps2m, \
                 tc.tile_pool(name="psm3", bufs=1, space="PSUM") as psm3:
                psH1 = [psm3.tile([128, 512], F32, tag=f"psH1{j}",
                                  name=f"psH1{j}") for j in range(4)]
                for g in range(12):
                    wt2 = wpool.tile([128, 4096], F16, tag="w", name="w").rearrange(
                        "p (a b c) -> p a b c", a=4, b=8)
                    nc.sync.dma_start(wt2[:], d_w2[g])
                    wt3 = wpool.tile([128, 4096], F16, tag="w", name="w").rearrange(
                        "p (i k) -> p i k", i=8)
                    nc.sync.dma_start(wt3[:], d_w3[g])
                    psS = ps2m.tile([128, 8, BS], F32, tag="psS")
                    for Ki in range(8):
                        K = 8 * g + Ki
                        for kk in range(4):
                            nc.tensor.matmul(psS[:, Ki, :],
                                             wt2[:, kk, Ki, :],
                                             hT[:, kk, :],
                                             start=(kk == 0), stop=False)
                        nc.tensor.matmul(psS[:, Ki, :],
                                         bbig[0:1, 128 * K:128 * (K + 1)],
                                         ones1[:], start=False, stop=True)
                    if g % 2 == 0:
                        nc.scalar.copy(specT[:, 8 * g:8 * (g + 1), :],
                                       psS[:, :, :])
                    else:
                        nc.vector.tensor_copy(specT[:, 8 * g:8 * (g + 1), :],
                                              psS[:, :, :])
                    for i in range(8):
                        K = 8 * g + i
                        for j in range(4):
                            nc.tensor.matmul(
                                psH1[j][:, 0:BS],
                                wt3[:, i, 128 * j:128 * (j + 1)],
                                specT[:, K, :],
                                start=(K == 0), stop=(K == NK - 1))
                for j in range(4):
                    nc.scalar.activation(h1T[:, j, :], psH1[j][:, 0:BS],
                                         AF.Relu, bias=b3s[:, j:j + 1])

            nc.gpsimd.dma_start(bbig[:], d_b5[:])

            # ======= L4: 512 -> 512, single weight tile =====================
            with tc.tile_pool(name="psm4", bufs=1, space="PSUM") as psm4:
                w4 = wp4.tile([128, 4, 4, 128], F16, tag="w4")
                nc.sync.dma_start(w4[:], d_w4[:])
                psH2 = psm4.tile([128, 4, BS], F32, tag="psH2")
                for j in range(4):
                    for kk in range(4):
                        nc.tensor.matmul(psH2[:, j, :],
                                         w4[:, kk, j, :],
                                         h1T[:, kk, :],
                                         start=(kk == 0), stop=(kk == 3))
                    nc.scalar.activation(h2T[:, j, :], psH2[:, j, :], AF.Relu,
                                         bias=b4s[:, j:j + 1])

            # ======= L5 + ifft2, emission-interleaved by channel ============
            # psum chunk K = (c = K//32, v-pair): partition q = 64*(v&1) + u
            # lands psP[0:64] -> even v columns, psP[64:128] -> odd v.
            with tc.tile_pool(name="gpi", bufs=6) as gpi, \
                 tc.tile_pool(name="opool", bufs=6) as opool, \
                 tc.tile_pool(name="ps5m", bufs=2, space="PSUM") as ps5m, \
                 tc.tile_pool(name="ps1i", bufs=4, space="PSUM") as ps1i, \
                 tc.tile_pool(name="ps2i", bufs=2, space="PSUM") as ps2i:

                ot_box = [None]

                def ifft2_bg(c, bg, flush=False):
                    psA = ps1i.tile([64, 512], F32, tag="psAi", name="psAi")
                    for t in range(4):
                        b = bg * 4 + t
                        nc.tensor.matmul(psA[:, t * 128:(t + 1) * 128],
                                         procTs[c][:, :, b],
                                         cswi[:], start=True, stop=True)
                    g1w = gpi.tile([64, 4, 128], F16, tag="g1i", name="g1i")
                    if bg % 2 == 0:
                        nc.scalar.copy(g1w.rearrange("p a k -> p (a k)"),
                                       psA[:])
                    else:
                        nc.vector.tensor_copy(
                            g1w.rearrange("p a k -> p (a k)"), psA[:])
                    ps2 = ps2i.tile([64, 256], F32, tag="p2i", name="p2i")
                    nc.tensor.matmul(ps2[:], cmi[:], g1w[:, :, 0:64],
                                     start=True, stop=False)
                    nc.tensor.matmul(ps2[:], smi[:], g1w[:, :, 64:128],
                                     start=False, stop=True)
                    # two bg's share one fp16 staging tile -> one DMA;
                    # staging copies ride the otherwise-idle gpsimd engine,
                    # and the post-stream flush DMAs use the freed SP queue.
                    if bg % 2 == 0:
                        ot_box[0] = opool.tile([64, 2, 256], F16, tag="ot",
                                               name="ot")
                        nc.vector.tensor_copy(ot_box[0][:, 0, :], ps2[:])
                    else:
                        nc.scalar.copy(ot_box[0][:, 1, :], ps2[:])
                        eng = nc.sync if flush else nc.gpsimd
                        eng.dma_start(
                            d_out[:, (c * 8 + bg - 1) * 256:
                                  (c * 8 + bg + 1) * 256],
                            ot_box[0].rearrange("p a k -> p (a k)"))

                # ifft2 image-groups of channel c are spread 2-per-slab
                # across the next channel's L5 slabs so their copy waits
                # overlap slab matmuls; the last channel flushes at the end.
                pending = []
                for g in range(12):
                    c, gc = g // 4, g % 4
                    wt5 = wpool.tile([128, 4096], F16, tag="w", name="w").rearrange(
                        "p (a b c) -> p a b c", a=4, b=8)
                    nc.sync.dma_start(wt5[:], d_w5[g])
                    psP = ps5m.tile([128, 8, BS], F32, tag="psP")
                    for Ki in range(8):
                        K = 8 * g + Ki
                        for kk in range(4):
                            nc.tensor.matmul(psP[:, Ki, :],
                                             wt5[:, kk, Ki, :],
                                             h2T[:, kk, :],
                                             start=(kk == 0), stop=False)
                        nc.tensor.matmul(psP[:, Ki, :],
                                         bbig[0:1, 128 * K:128 * (K + 1)],
                                         ones1[:], start=False, stop=True)
                    v0 = gc * 16
                    if g % 2 == 0:
                        nc.scalar.copy(procTs[c][:, v0:v0 + 16:2, :],
                                       psP[0:64, :, :])
                        nc.vector.tensor_copy(
                            procTs[c][:, v0 + 1:v0 + 16:2, :],
                            psP[64:128, :, :])
                    else:
                        nc.vector.tensor_copy(procTs[c][:, v0:v0 + 16:2, :],
                                              psP[0:64, :, :])
                        nc.scalar.copy(procTs[c][:, v0 + 1:v0 + 16:2, :],
                                       psP[64:128, :, :])
                    for cb in pending[:3]:
                        ifft2_bg(*cb)
                    pending = pending[3:]
                    if gc == 3:
                        pending += [(c, bg) for bg in range(BS // 4)]
                for cb in pending:
                    ifft2_bg(*cb, flush=True)
                if dbg:
                    for t, n in ((hT, "hT"), (h1T, "h1T"), (h2T, "h2T"),
                                 (specT, "specT"), (procTs[0], "procT0"),
                                 (frlin, "frlin")):
                        nc.scalar.dma_start(d_dbg[n][:], t[:])

    nc.compile()
    return nc


def _get_nc():
    global _NC_CACHE
    if _NC_CACHE is None:
        _NC_CACHE = _build_nc()
    return _NC_CACHE


def _make_in_maps(x, Ws1, bs1, Ws2, bs2, Wn1, bn1, Wn2, bn2, Wn3, bn3):
    shared = dict(_host_constants())
    shared.update(_prep_weights(Ws1, bs1, Ws2, bs2, Wn1, bn1, Wn2, bn2,
                                Wn3, bn3))
    in_maps = []
    for g in range(NCORE):
        if g == 0:
            halo = np.zeros((1, 64, 64), np.float32)
        else:
            halo = x[g * BS - 1, 2][None]
        ximgs = np.concatenate(
            [halo, x[g * BS:(g + 1) * BS].reshape(NIMG, 64, 64),
             np.zeros((1, 64, 64), np.float32)]).astype(np.float16)
        # pre-transpose to [u, (slot, v)] so the device DMA is contiguous
        xT = np.ascontiguousarray(ximgs.transpose(1, 0, 2).reshape(64, -1))
        in_maps.append({"ximgs": xT, **shared})
    return in_maps


def _assemble(per_core_outs):
    """per_core_outs[g]: [64, NIMG*64] f32; row = v', col = (c*32 + b)*64 + u'
    (procT is [u-part, v-free], so the ifft2 DFT pair contracts u first and
    stage-2 psum comes out image-transposed)."""
    out = np.empty((B, C, H, W), np.float32)
    for g in range(NCORE):
        a = np.asarray(per_core_outs[g]).astype(np.float32).reshape(
            64, C, BS, 64)
        out[g * BS:(g + 1) * BS] = a.transpose(2, 1, 3, 0)
    return out


def kernel(**inputs):
    x = np.ascontiguousarray(inputs["x"], dtype=np.float32)
    nc = _get_nc()
    in_maps = _make_in_maps(
        x, inputs["Ws1"], inputs["bs1"], inputs["Ws2"], inputs["bs2"],
        inputs["Wn1"], inputs["bn1"], inputs["Wn2"], inputs["bn2"],
        inputs["Wn3"], inputs["bn3"])
    res = run_bass_kernel_spmd(nc, in_maps, list(range(NCORE)))
    return _assemble([res.results[g]["out"] for g in range(NCORE)])
